# revision 6
# baseline (speedup 1.0000x reference)
"""Additive attention (B=16, Q=128, K=1024, D=256, H=64) on 8 trn2 NeuronCores.

Strategy
--------
scores[b,q,k] = sum_h Wv[h] * tanh(qproj[b,q,h] + kproj[b,k,h]); softmax over
valid k only; out = attn @ values.  Positions k >= valid_len get softmax weight
exactly 0 in fp32, so they are skipped / masked to zero.

Work unit = (batch, half-of-Q) -> 64 queries.  32 units are sorted by
valid_len (descending, host side: valid_lens is known at kernel-build time)
and processed in 4 "slots" of 8 units; the 8 units of a slot run on the 8
cores in parallel with a common compile-time K extent (the slot max).  Units
whose valid_len is below the slot K mask the surplus keys with a 0/1
per-partition multiplier fused into the attn-transpose copy.

Per (core, slot):
  - host supplies keysT [256,Ks] f32, values_aug [KC*128, 258] bf16 (col 256
    = ones so the attention-weight row-sum falls out of the AV matmul),
    queriesT [256,64] f32.
  - PE: kprojT duplicated to both 64-partition halves -> psum; DVE copies to
    bf16 sbuf [128, Ks]   (row r = 64*j + h holds kproj[.,h], j=0,1)
  - PE: qproj packed [128, 32]: row 64*j+h, col p = qproj[2p+j, h]
  - DVE tensor_scalar_add (bf16, 4x mode): feat[:, p, :] = kp + qp[:, p]
  - ACT: tanh over [128, 8*Ks] groups (the engine floor: 1 elem/cycle/lane)
  - PE: scores accumulate: for each pack p, lhsT = Wv embedded in columns
    2p,2p+1 of a [128,64] zero matrix -> psum [64, <=512] accumulates all 32
  - ACT: exp (no max-subtraction needed: |score| <= sum|Wv|, host-checked)
  - PE transpose per 128-k chunk + DVE masked copy -> attnT bf16
  - PE: attnT @ values_aug accumulate -> [64, 258]; col 256 = sum of weights
  - DVE: out = av[:, :256] * reciprocal(av[:, 256])
"""

import sys

for _p in ("/opt/trn_rl_repo",):
    if _p not in sys.path:
        sys.path.append(_p)

import numpy as np
import ml_dtypes

import concourse.bass as bass  # noqa: F401  (bass types used via tile/bacc)
import concourse.tile as tile
from concourse import bacc, mybir
from concourse.bass_utils import run_bass_kernel_spmd

F32 = mybir.dt.float32
BF16 = mybir.dt.bfloat16
BF = ml_dtypes.bfloat16

B, Q, K, D, H, V = 16, 128, 1024, 256, 64, 256
VW = 258          # 256 values + ones column + pad
NCORES = 8
QCH = 64          # queries per unit
PACKS = QCH // 2  # q-pairs per unit
GS = 8            # packs per tanh group
NSLOTS = (B * (Q // QCH)) // NCORES

_cache = {}


def _build(ks_list, exp_shift):
    """Build the SPMD program for per-slot K extents ks_list."""
    nc = bacc.Bacc("TRN2", target_bir_lowering=False, debug=False,
                   num_devices=NCORES)
    kcs = [(ks + 127) // 128 for ks in ks_list]
    colbase = [sum(kcs[:j]) for j in range(len(kcs))]
    nch = sum(kcs)

    kT_d = [nc.dram_tensor(f"kT{j}", [D, ks], F32, kind="ExternalInput")
            for j, ks in enumerate(ks_list)]
    vA_d = [nc.dram_tensor(f"vA{j}", [kc * 128, VW], BF16, kind="ExternalInput")
            for j, kc in enumerate(kcs)]
    qT_d = [nc.dram_tensor(f"qT{j}", [D, QCH], F32, kind="ExternalInput")
            for j in range(NSLOTS)]
    wkT_d = nc.dram_tensor("wkT", [D, H], F32, kind="ExternalInput")
    wqT_d = nc.dram_tensor("wqT", [D, H], F32, kind="ExternalInput")
    wvs_d = nc.dram_tensor("wvs", [128, PACKS * QCH], BF16, kind="ExternalInput")
    id_d = nc.dram_tensor("id64", [H, H], F32, kind="ExternalInput")
    vm_d = nc.dram_tensor("vmask", [128, nch], F32, kind="ExternalInput")
    out_d = nc.dram_tensor("out", [NSLOTS, QCH, V], F32, kind="ExternalOutput")

    with tile.TileContext(nc) as tc:
        with (
            tc.tile_pool(name="const", bufs=1) as const,
            tc.tile_pool(name="sb_k", bufs=2) as sb_k,
            tc.tile_pool(name="sb_v", bufs=2) as sb_v,
            tc.tile_pool(name="sb_q", bufs=2) as sb_q,
            tc.tile_pool(name="sb_kp", bufs=2) as sb_kp,
            tc.tile_pool(name="sb_feat", bufs=2) as sb_feat,
            tc.tile_pool(name="sb_tanh", bufs=2) as sb_tanh,
            tc.tile_pool(name="sb_attn", bufs=2) as sb_attn,
            tc.tile_pool(name="sb_aT", bufs=4) as sb_aT,
            tc.tile_pool(name="sb_out", bufs=2) as sb_out,
            tc.tile_pool(name="ps_kp", bufs=2, space="PSUM") as ps_kp,
            tc.tile_pool(name="ps_sc", bufs=2, space="PSUM") as ps_sc,
            tc.tile_pool(name="ps_sm", bufs=2, space="PSUM") as ps_sm,
            tc.tile_pool(name="ps_av", bufs=2, space="PSUM") as ps_av,
        ):
            wk_sb = const.tile([128, 2, H], F32)
            nc.sync.dma_start(out=wk_sb, in_=wkT_d.ap().rearrange(
                "(c p) h -> p c h", p=128))
            wq_sb = const.tile([128, 2, H], F32)
            nc.sync.dma_start(out=wq_sb, in_=wqT_d.ap().rearrange(
                "(c p) h -> p c h", p=128))
            wvs_sb = const.tile([128, PACKS, QCH], BF16)
            nc.sync.dma_start(out=wvs_sb, in_=wvs_d.ap().rearrange(
                "p (k m) -> p k m", k=PACKS))
            id_sb = const.tile([H, H], F32)
            nc.sync.dma_start(out=id_sb, in_=id_d.ap())
            vm_sb = const.tile([128, nch], F32)
            nc.sync.dma_start(out=vm_sb, in_=vm_d.ap())

            for j, ks in enumerate(ks_list):
                kc = kcs[j]
                sc_chunks = [(s, min(512, ks - s)) for s in range(0, ks, 512)]

                kt = sb_k.tile([128, 2, ks], F32, tag="kt")
                nc.sync.dma_start(out=kt, in_=kT_d[j].ap().rearrange(
                    "(c p) k -> p c k", p=128))
                vt = sb_v.tile([128, kc, VW], BF16, tag="vt")
                nc.sync.dma_start(out=vt, in_=vA_d[j].ap().rearrange(
                    "(c p) v -> p c v", p=128))
                qt = sb_q.tile([128, 2, QCH], F32, tag="qt")
                nc.sync.dma_start(out=qt, in_=qT_d[j].ap().rearrange(
                    "(c p) q -> p c q", p=128))

                # ---- qproj packed [128, PACKS]
                qp_sb = sb_q.tile([128, PACKS], F32, tag="qp")
                for par in (0, 1):
                    qp_ps = ps_sm.tile([64, PACKS], F32, tag="sm",
                                       name=f"qp_ps{j}_{par}")
                    for dc in (0, 1):
                        nc.tensor.matmul(
                            qp_ps[:, :],
                            wq_sb[:, dc, :],
                            qt[:, dc, par::2],
                            start=(dc == 0), stop=(dc == 1))
                    nc.vector.tensor_copy(qp_sb[64 * par:64 * par + 64, :], qp_ps)

                # ---- kprojT duplicated -> bf16 [128, ks]
                kp = sb_kp.tile([128, ks], BF16, tag="kp")
                for s0, cw in sc_chunks:
                    kp_ps = ps_kp.tile([64, cw], F32, tag="kp")
                    for dc in (0, 1):
                        nc.tensor.matmul(
                            kp_ps[:, :],
                            wk_sb[:, dc, :],
                            kt[:, dc, s0:s0 + cw],
                            start=(dc == 0), stop=(dc == 1))
                    for dup in (0, 1):
                        nc.vector.tensor_copy(
                            kp[64 * dup:64 * dup + 64, s0:s0 + cw], kp_ps)

                # ---- features -> tanh -> scores
                sc_tiles = [ps_sc.tile([QCH, cw], F32, tag="sc", name=f"sc{j}_{ci}")
                            for ci, (s0, cw) in enumerate(sc_chunks)]
                ngroups = PACKS // GS
                for g in range(ngroups):
                    feat = sb_feat.tile([128, GS, ks], BF16, tag="feat")
                    tanhg = sb_tanh.tile([128, GS, ks], BF16, tag="tanh")
                    for p8 in range(GS):
                        p = g * GS + p8
                        nc.vector.tensor_scalar_add(
                            feat[:, p8, :], kp[:, :], qp_sb[:, p:p + 1])
                    nc.scalar.activation(
                        tanhg[:, :, :], feat[:, :, :],
                        mybir.ActivationFunctionType.Tanh)
                    for p8 in range(GS):
                        p = g * GS + p8
                        for ci, (s0, cw) in enumerate(sc_chunks):
                            nc.tensor.matmul(
                                sc_tiles[ci][:, :],
                                wvs_sb[:, p, :],
                                tanhg[:, p8, s0:s0 + cw],
                                start=(p == 0), stop=(p == PACKS - 1))

                # ---- exp (no max subtraction; optional constant shift)
                attn = sb_attn.tile([QCH, ks], F32, tag="attn")
                for ci, (s0, cw) in enumerate(sc_chunks):
                    nc.scalar.activation(
                        attn[:, s0:s0 + cw], sc_tiles[ci][:, :],
                        mybir.ActivationFunctionType.Exp,
                        bias=-exp_shift)

                # ---- transpose + mask + AV matmul
                av_ps = ps_av.tile([QCH, VW], F32, tag="av")
                for t in range(kc):
                    c0 = 128 * t
                    cc = min(128, ks - c0)
                    tr = ps_sm.tile([128, H], F32, tag="sm", name=f"tr{j}_{t}")
                    nc.tensor.transpose(tr[:cc, :], attn[:, c0:c0 + cc], id_sb)
                    aT = sb_aT.tile([128, H], BF16, tag="aT")
                    nc.vector.tensor_scalar_mul(
                        aT[:cc, :], tr[:cc, :],
                        vm_sb[:cc, colbase[j] + t:colbase[j] + t + 1])
                    nc.tensor.matmul(
                        av_ps[:, :], aT[:cc, :], vt[:cc, t, :],
                        start=(t == 0), stop=(t == kc - 1))

                # ---- normalize + store
                rcp = sb_out.tile([QCH, 1], F32, tag="rcp")
                nc.vector.reciprocal(rcp, av_ps[:, V:V + 1])
                outt = sb_out.tile([QCH, V], F32, tag="out")
                nc.vector.tensor_scalar_mul(outt, av_ps[:, 0:V], rcp)
                nc.sync.dma_start(out=out_d.ap()[j], in_=outt)

    nc.compile()
    return nc


def _prep(queries, keys, values, valid_lens, Wq, Wk, Wv):
    """Host-side sharding/layout prep. Returns (signature pieces, in_maps,
    assignment) used by kernel()."""
    vl = [int(x) for x in np.asarray(valid_lens).reshape(-1)]
    assert len(vl) == B
    units = sorted(
        [(vl[b], b, h) for b in range(B) for h in (0, 1)],
        key=lambda u: -u[0])
    # slot j, core c -> units[8*j + c]; slot K extent = max vl in slot
    ks_list = [units[NCORES * j][0] for j in range(NSLOTS)]
    kcs = [(ks + 127) // 128 for ks in ks_list]
    nch = sum(kcs)

    qT = np.ascontiguousarray(np.transpose(np.asarray(queries, np.float32),
                                           (0, 2, 1)))          # [B, D, Q]
    kT = np.ascontiguousarray(np.transpose(np.asarray(keys, np.float32),
                                           (0, 2, 1)))          # [B, D, K]
    va = np.zeros((B, K, VW), BF)
    va[:, :, :V] = np.asarray(values, BF)
    va[:, :, V] = BF(1.0)

    wkT = np.ascontiguousarray(np.asarray(Wk, np.float32).T)    # [D, H]
    wqT = np.ascontiguousarray(np.asarray(Wq, np.float32).T)    # [D, H]
    wv = np.asarray(Wv, np.float32).reshape(-1)                 # [H]
    # score bound -> optional constant shift inside exp (cancels in softmax)
    bound = float(np.abs(wv).sum())
    exp_shift = max(0.0, bound - 30.0)

    wvs = np.zeros((128, PACKS * QCH), BF)
    wvb = wv.astype(BF)
    for p in range(PACKS):
        for par in (0, 1):
            wvs[64 * par:64 * par + 64, p * QCH + 2 * p + par] = wvb
    id64 = np.eye(H, dtype=np.float32)

    in_maps = []
    assignment = []
    for c in range(NCORES):
        m = {"wkT": wkT, "wqT": wqT, "wvs": wvs, "id64": id64}
        vm = np.zeros((128, nch), np.float32)
        amap = []
        base = 0
        for j in range(NSLOTS):
            myvl, b, h = units[NCORES * j + c]
            ks, kc = ks_list[j], kcs[j]
            amap.append((b, h))
            m[f"kT{j}"] = np.ascontiguousarray(kT[b, :, :ks])
            m[f"vA{j}"] = np.ascontiguousarray(va[b, :kc * 128, :])
            m[f"qT{j}"] = np.ascontiguousarray(
                qT[b, :, h * QCH:(h + 1) * QCH])
            k_idx = np.arange(128)[:, None] + 128 * np.arange(kc)[None, :]
            vm[:, base:base + kc] = (k_idx < myvl).astype(np.float32)
            base += kc
        m["vmask"] = vm
        in_maps.append(m)
        assignment.append(amap)
    return tuple(ks_list), exp_shift, in_maps, assignment


def kernel(queries, keys, values, valid_lens, Wq, Wk, Wv):
    ks_list, exp_shift, in_maps, assignment = _prep(
        queries, keys, values, valid_lens, Wq, Wk, Wv)
    key = (ks_list, round(exp_shift, 3))
    if key not in _cache:
        _cache[key] = _build(list(ks_list), exp_shift)
    nc = _cache[key]
    res = run_bass_kernel_spmd(nc, in_maps, list(range(NCORES)))
    out = np.zeros((B, Q, V), np.float32)
    for c in range(NCORES):
        o = res.results[c]["out"]           # [NSLOTS, QCH, V]
        for j, (b, h) in enumerate(assignment[c]):
            out[b, h * QCH:(h + 1) * QCH, :] = o[j]
    return out


if __name__ == "__main__":
    # quick CoreSim correctness check on core 0's program
    from concourse.bass_interp import CoreSim

    rng = np.random.default_rng(0)
    queries = rng.standard_normal((B, Q, D), np.float32)
    keys = rng.standard_normal((B, K, D), np.float32)
    values = rng.standard_normal((B, K, V), np.float32)
    valid_lens = rng.integers(1, K + 1, (B,)).astype(np.int64)
    Wq = (rng.standard_normal((H, D), np.float32) / np.sqrt(D)).astype(np.float32)
    Wk = (rng.standard_normal((H, D), np.float32) / np.sqrt(D)).astype(np.float32)
    Wv = (rng.standard_normal((1, H), np.float32) / np.sqrt(H)).astype(np.float32)

    ks_list, exp_shift, in_maps, assignment = _prep(
        queries, keys, values, valid_lens, Wq, Wk, Wv)
    print("ks_list:", ks_list, "exp_shift:", exp_shift)
    nc = _build(list(ks_list), exp_shift)
    print("built+compiled")

    sim = CoreSim(nc, trace=False)
    for name, arr in in_maps[0].items():
        sim.tensor(name)[:] = arr
    sim.simulate()
    got = np.array(sim.tensor("out"))

    # numpy reference for core 0's units
    q = queries @ Wq.T
    k = keys @ Wk.T
    for j, (b, h) in enumerate(assignment[0]):
        feats = np.tanh(q[b, h * QCH:(h + 1) * QCH, None, :] + k[b, None, :, :])
        scores = feats @ Wv[0]
        vlb = int(valid_lens[b])
        scores[:, vlb:] = -1e6
        e = np.exp(scores - scores.max(-1, keepdims=True))
        attn = e / e.sum(-1, keepdims=True)
        exp_out = attn @ values[b]
        err = np.abs(got[j] - exp_out)
        rel = err.max() / np.abs(exp_out).max()
        print(f"slot {j} (b={b},h={h}, vl={vlb}): absmax-rel err {rel:.3e}")
